# revision 6
# baseline (speedup 1.0000x reference)
"""Trainium2 Bass kernel for a Swin-style transformer block
(windowed attention with RoPE + SwiGLU MLP with sub-LN).

Sharding: data-parallel over batch B=8 -> one image per NeuronCore.
Each core computes the full block for its image in window-partitioned,
feature-major layout; the host does window (un)partitioning, LN-affine
folding into the projection weights, and RoPE table generation.

v2: vector-engine offload. LN statistics via PE ones-matmuls on bf16
copies (squares on ScalarE); LN centering folded into the projection
matmuls as a rank-1 correction (lhsT = -colsum(W)/n, rhs = CA row);
LN scaling folded into the matmul *input* (xA = xb * Ab); RoPE via two
accumulating matmuls (identity + rotation); softmax over head-pairs
(two heads per PSUM tile); SwiGLU via ScalarE Silu. PSUM evacuations
on ScalarE. This balances TensorE ~ VectorE ~ ScalarE instead of the
v1 vector-bound profile.
"""
import numpy as np
import ml_dtypes
from contextlib import ExitStack

import concourse.bass as bass
import concourse.tile as tile
from concourse import bacc, mybir
from concourse.bass_utils import run_bass_kernel_spmd

BF16NP = ml_dtypes.bfloat16
F32 = mybir.dt.float32
BF16 = mybir.dt.bfloat16
OP = mybir.AluOpType
AF = mybir.ActivationFunctionType

DIM = 768
HEADS = 12
HD = 64
HID = 2048
EPS = 1e-6
WS = 14
NTOK = WS * WS          # 196 tokens per window
B, H, W = 8, 64, 64
NWIN = 25               # 5x5 windows per image
TOKS = NWIN * NTOK      # 4900
KT = DIM // 128         # 6 feature tiles
MT = HID // 128         # 16 hid tiles
N_CORES = 8
P = 128
PC2 = 2 * NTOK          # 392: max columns per window-pair
CHUNKS = [(0, P), (P, NTOK - P)]   # [128, 68] token chunks per window

_cache = {}


def _rope_tables():
    dim, pt, theta = 32, 16.0, 10000.0
    freqs = 1.0 / theta ** (np.arange(0, dim, 2, dtype=np.float32) / dim)
    f1 = np.repeat((np.arange(WS, dtype=np.float32) / WS * pt)[:, None] * freqs[None, :], 2, axis=-1)
    f = np.concatenate([
        np.broadcast_to(f1[:, None, :], (WS, WS, dim)),
        np.broadcast_to(f1[None, :, :], (WS, WS, dim)),
    ], -1).reshape(NTOK, 2 * dim)
    return np.cos(f), np.sin(f)   # [196, 64] fp32


def _bcast_ap(row_ap, nrep):
    """AP that replicates a [1, n] row across nrep partitions."""
    return bass.AP(tensor=row_ap.tensor, offset=row_ap.offset,
                   ap=[row_ap.ap[0], [0, nrep], row_ap.ap[1]])


def _emit(nc, tc, ctx, aps, has_biases, nwin_total=NWIN, loop_n=1):
    pairs = []
    w = 0
    while w < nwin_total:
        pairs.append((w, w + 1) if w + 1 < nwin_total else (w,))
        w += 2

    xTb = aps["xTb"].rearrange("(k p) n -> p k n", p=P)   # [128, 6, TOKS] bf16
    yT = aps["yT"].rearrange("(k p) n -> p k n", p=P)
    w3d = aps["w3"].rearrange("(k p) m -> p k m", p=P)    # [128, 16, 768]

    consts = ctx.enter_context(tc.tile_pool(name="consts", bufs=1))
    wpool = ctx.enter_context(tc.tile_pool(name="weights", bufs=1))
    w3pool = ctx.enter_context(tc.tile_pool(name="w3s", bufs=2))
    xpool = ctx.enter_context(tc.tile_pool(name="x", bufs=2))
    bfpool = ctx.enter_context(tc.tile_pool(name="bf", bufs=1))
    rowpool = ctx.enter_context(tc.tile_pool(name="rows", bufs=2))
    abpool = ctx.enter_context(tc.tile_pool(name="ab", bufs=1))
    qspool = ctx.enter_context(tc.tile_pool(name="qsp", bufs=2))
    qkpool = ctx.enter_context(tc.tile_pool(name="qk", bufs=1))
    vpool = ctx.enter_context(tc.tile_pool(name="v", bufs=1))
    epool = ctx.enter_context(tc.tile_pool(name="e", bufs=2))
    zpool = ctx.enter_context(tc.tile_pool(name="z", bufs=2))
    opool = ctx.enter_context(tc.tile_pool(name="oh", bufs=1))
    x1pool = ctx.enter_context(tc.tile_pool(name="x1", bufs=1))
    mlppool = ctx.enter_context(tc.tile_pool(name="mlp", bufs=2))
    gpool = ctx.enter_context(tc.tile_pool(name="g", bufs=1))
    ypool = ctx.enter_context(tc.tile_pool(name="y", bufs=2))

    ps_mm = ctx.enter_context(tc.tile_pool(name="psmm", bufs=2, space="PSUM"))
    ps_att = ctx.enter_context(tc.tile_pool(name="psatt", bufs=2, space="PSUM"))
    ps_stat = ctx.enter_context(tc.tile_pool(name="psstat", bufs=2, space="PSUM"))

    # --- constants / weights in SBUF ---
    def load_w(name, kdim, mdim):
        t = wpool.tile([P, kdim // P, mdim], BF16, tag=name)
        nc.sync.dma_start(t[:], aps[name].rearrange("(k p) m -> p k m", p=P))
        return t

    wq = load_w("wq", DIM, DIM)
    wk = load_w("wk", DIM, DIM)
    wv = load_w("wv", DIM, DIM)
    wp = load_w("wp", DIM, DIM)
    w1 = load_w("w1", DIM, HID)
    w2 = load_w("w2", DIM, HID)

    def load_row(name, n):
        if aps.get(name) is None:
            return None
        t = consts.tile([1, n], BF16, tag=name)
        nc.sync.dma_start(t[:], aps[name][:])
        return t

    nqs = load_row("nqs", DIM)
    nks = load_row("nks", DIM)
    nvs = load_row("nvs", DIM)
    nw1s = load_row("nw1s", HID)
    nw2s = load_row("nw2s", HID)
    nw3s = load_row("nw3s", DIM)
    qbr = load_row("qbr", DIM)
    kbr = load_row("kbr", DIM)
    vbr = load_row("vbr", DIM)

    cos2 = consts.tile([P, PC2], BF16, tag="cos2")
    nc.sync.dma_start(cos2[:], aps["cos2"][:])
    sin2 = consts.tile([P, PC2], BF16, tag="sin2")
    nc.sync.dma_start(sin2[:], aps["sin2"][:])
    r2t = consts.tile([P, P], BF16, tag="r2t")
    nc.sync.dma_start(r2t[:], aps["r2t"][:])
    ident = consts.tile([P, P], BF16, tag="ident")
    nc.sync.dma_start(ident[:], aps["ident"][:])
    ones = consts.tile([P, 1], BF16, tag="ones")
    nc.vector.memset(ones[:], 1.0)
    ones_f = consts.tile([P, 1], F32, tag="ones_f")
    nc.vector.memset(ones_f[:], 1.0)
    ones_row = consts.tile([1, PC2], BF16, tag="ones_row")
    nc.vector.memset(ones_row[:], 1.0)
    zcol = consts.tile([P, 1], F32, tag="zcol")
    nc.vector.memset(zcol[:], 0.0)
    zrow1 = consts.tile([1, 1], F32, tag="zrow1")
    nc.vector.memset(zrow1[:], 0.0)
    epsr = consts.tile([1, 1], F32, tag="epsr")
    nc.vector.memset(epsr[:], EPS)

    def bias_col(name, feat):
        if aps.get(name) is None:
            return None
        t = consts.tile([P, feat // P], F32, tag=name)
        nc.sync.dma_start(t[:], aps[name].rearrange("(k p) -> p k", p=P))
        return t

    pb = bias_col("pb", DIM)
    w1b = bias_col("w1b", HID)
    w2b = bias_col("w2b", HID)
    w3b = bias_col("w3b", DIM)
    vbc = bias_col("vbc", DIM)

    def sc(bcol, m):
        return 0.0 if bcol is None else bcol[:, m:m + 1]

    def ln_stats(xf_t, pc):
        """xf_t [P, KT, pc] fp32 -> (sum, sumsq) PSUM rows [1, pc] fp32."""
        sxp = ps_stat.tile([1, PC2], F32, tag="srow")
        sqp = ps_stat.tile([1, PC2], F32, tag="srow")
        ones_l = ones_f if xf_t.dtype == F32 else ones
        for k in range(KT):
            nc.tensor.matmul(sxp[:, :pc], lhsT=ones_l[:, 0:1], rhs=xf_t[:, k, :pc],
                             start=(k == 0), stop=(k == KT - 1), skip_group_check=True)
        for k in range(KT):
            sqt = mlppool.tile([P, PC2], BF16, tag="sqt")
            nc.scalar.activation(out=sqt[:, :pc], in_=xf_t[:, k, :pc],
                                 func=AF.Square, bias=zcol[:], scale=1.0)
            nc.tensor.matmul(sqp[:, :pc], lhsT=ones[:, 0:1], rhs=sqt[:, :pc],
                             start=(k == 0), stop=(k == KT - 1), skip_group_check=True)
        return sxp, sqp

    def ln_tail(sxp, sqp, nfeat, pc, abtag, want_ca=True):
        """-> (A row bf16, CA=sx*A row bf16, Ab [128,pc] bf16 broadcast)."""
        sx2 = rowpool.tile([1, PC2], F32, tag="sx2")
        nc.scalar.activation(out=sx2[:, :pc], in_=sxp[:, :pc],
                             func=AF.Square, bias=zrow1[:], scale=1.0)
        tv = rowpool.tile([1, PC2], F32, tag="tv")
        nc.vector.scalar_tensor_tensor(out=tv[:, :pc], in0=sqp[:, :pc],
                                       scalar=float(nfeat), in1=sx2[:, :pc],
                                       op0=OP.mult, op1=OP.subtract)
        sig = rowpool.tile([1, PC2], F32, tag="sig")
        nc.scalar.activation(out=sig[:, :pc], in_=tv[:, :pc], func=AF.Sqrt,
                             bias=epsr[:], scale=1.0 / float(nfeat) ** 2)
        A = rowpool.tile([1, PC2], BF16, tag="arow")
        with nc.allow_low_precision(reason="LN scale in bf16"):
            nc.vector.reciprocal(out=A[:, :pc], in_=sig[:, :pc])
        CA = None
        if want_ca:
            CA = rowpool.tile([1, PC2], BF16, tag="carow")
            nc.vector.tensor_tensor(out=CA[:, :pc], in0=sxp[:, :pc],
                                    in1=A[:, :pc], op=OP.mult)
        Ab = abpool.tile([P, PC2], BF16, tag=abtag)
        nc.sync.dma_start(Ab[:, :pc], _bcast_ap(A[:, :pc], P))
        return A, CA, Ab

    def emit_pair(wins):
        nwin = len(wins)
        pc = NTOK * nwin
        c0 = wins[0] * NTOK

        xb = xpool.tile([P, KT, PC2], BF16, tag="x")
        nc.sync.dma_start(xb[:, :, :pc], xTb[:, :, c0:c0 + pc])

        # ---------- LN1: stats via PE, scale folded into inputs ----------
        sxp, sqp = ln_stats(xb, pc)
        A1, CA1, Ab1 = ln_tail(sxp, sqp, DIM, pc, "ab1")
        xA = bfpool.tile([P, KT, PC2], BF16, tag="xA")
        for k in range(KT):
            nc.vector.tensor_tensor(out=xA[:, k, :pc], in0=xb[:, k, :pc],
                                    in1=Ab1[:, :pc], op=OP.mult)

        # ---------- Q/K projections + RoPE ----------
        def emit_qk(wmat, nsum, brow, dst):
            for m in range(KT):
                psQ = ps_mm.tile([P, PC2], F32, tag="mm")
                for k in range(KT):
                    nc.tensor.matmul(psQ[:, :pc], lhsT=wmat[:, k, m * P:(m + 1) * P],
                                     rhs=xA[:, k, :pc], start=(k == 0), stop=False)
                nc.tensor.matmul(psQ[:, :pc], lhsT=nsum[0:1, m * P:(m + 1) * P],
                                 rhs=CA1[:, :pc], start=False, stop=(brow is None))
                if brow is not None:
                    nc.tensor.matmul(psQ[:, :pc], lhsT=brow[0:1, m * P:(m + 1) * P],
                                     rhs=ones_row[:, :pc], start=False, stop=True)
                qs = qspool.tile([P, PC2], BF16, tag="qs")
                nc.scalar.activation(out=qs[:, :pc], in_=psQ[:, :pc],
                                     func=AF.Copy, bias=0.0, scale=1.0)
                qc = qspool.tile([P, PC2], BF16, tag="qc")
                nc.vector.tensor_tensor(out=qc[:, :pc], in0=qs[:, :pc],
                                        in1=cos2[:, :pc], op=OP.mult)
                qn = qspool.tile([P, PC2], BF16, tag="qn")
                nc.vector.tensor_tensor(out=qn[:, :pc], in0=qs[:, :pc],
                                        in1=sin2[:, :pc], op=OP.mult)
                psR = ps_mm.tile([P, PC2], F32, tag="mm")
                nc.tensor.matmul(psR[:, :pc], lhsT=ident[:], rhs=qc[:, :pc],
                                 start=True, stop=False)
                nc.tensor.matmul(psR[:, :pc], lhsT=r2t[:], rhs=qn[:, :pc],
                                 start=False, stop=True)
                nc.scalar.activation(out=dst[:, m, :pc], in_=psR[:, :pc],
                                     func=AF.Copy, bias=0.0, scale=1.0)

        qhat = qkpool.tile([P, KT, PC2], BF16, tag="qhat")
        khat = qkpool.tile([P, KT, PC2], BF16, tag="khat")
        emit_qk(wq, nqs, qbr, qhat)
        emit_qk(wk, nks, kbr, khat)

        # ---------- V (token-major, per window, with ones column) ----------
        v_ts = []
        for wi in range(nwin):
            wcol = wi * NTOK
            vt = []
            for ci, (cs, cn) in enumerate(CHUNKS):
                v_t = vpool.tile([P, HEADS, HD + 1], BF16, tag=f"v{wi}{ci}")
                nc.vector.memset(v_t[:, :, HD:HD + 1], 1.0)
                for half in range(2):
                    nh = DIM // 2
                    vps = ps_mm.tile([P, nh], F32, tag="mm")
                    for k in range(KT):
                        nc.tensor.matmul(vps[0:cn, :],
                                         lhsT=xA[:, k, wcol + cs:wcol + cs + cn],
                                         rhs=wv[:, k, half * nh:(half + 1) * nh],
                                         start=(k == 0), stop=False)
                    nc.tensor.matmul(vps[0:cn, :],
                                     lhsT=CA1[0:1, wcol + cs:wcol + cs + cn],
                                     rhs=nvs[0:1, half * nh:(half + 1) * nh],
                                     start=False, stop=(vbr is None))
                    if vbr is not None:
                        nc.tensor.matmul(vps[0:cn, :],
                                         lhsT=ones_row[0:1, 0:cn],
                                         rhs=vbr[0:1, half * nh:(half + 1) * nh],
                                         start=False, stop=True)
                    nc.scalar.activation(
                        out=v_t[0:cn, half * (HEADS // 2):(half + 1) * (HEADS // 2), 0:HD],
                        in_=vps[0:cn, :].rearrange("p (h d) -> p h d", d=HD),
                        func=AF.Copy, bias=0.0, scale=1.0)
                vt.append(v_t)
            v_ts.append(vt)

        # ---------- attention: head-pairs (same row-parity, adjacent g) ----------
        ohat = opool.tile([P, KT, PC2], BF16, tag="ohat")
        for wi in range(nwin):
            wcol = wi * NTOK
            for b in range(2):
                r0 = 64 * b
                for j in range(3):
                    g0 = 2 * j
                    h0 = 2 * g0 + b
                    h1 = 2 * (g0 + 1) + b
                    es = []
                    for ci, (cs, cn) in enumerate(CHUNKS):
                        psS = ps_att.tile([P, PC2], F32, tag="s")
                        for cg, g in ((0, g0), (1, g0 + 1)):
                            nc.tensor.matmul(
                                psS[0:cn, cg * NTOK:(cg + 1) * NTOK],
                                lhsT=khat[r0:r0 + 64, g, wcol + cs:wcol + cs + cn],
                                rhs=qhat[r0:r0 + 64, g, wcol:wcol + NTOK],
                                start=True, stop=True, skip_group_check=True)
                        e = epool.tile([P, PC2], BF16, tag=f"e{ci}")
                        nc.scalar.activation(out=e[0:cn, :], in_=psS[0:cn, :],
                                             func=AF.Exp, bias=zcol[0:cn, :], scale=1.0)
                        es.append(e)
                    psO = ps_att.tile([P, PC2], F32, tag="o")
                    for cg, hh in ((0, h0), (1, h1)):
                        for ci, (cs, cn) in enumerate(CHUNKS):
                            nc.tensor.matmul(
                                psO[0:HD + 1, cg * NTOK:(cg + 1) * NTOK],
                                lhsT=v_ts[wi][ci][0:cn, hh, :],
                                rhs=es[ci][0:cn, cg * NTOK:(cg + 1) * NTOK],
                                start=(ci == 0), stop=(ci == 1),
                                skip_group_check=True)
                    zinv = zpool.tile([1, PC2], BF16, tag="zinv")
                    with nc.allow_low_precision(reason="softmax denom bf16"):
                        nc.vector.reciprocal(out=zinv[:], in_=psO[HD:HD + 1, :])
                    zb = zpool.tile([64, PC2], BF16, tag="zb")
                    nc.sync.dma_start(zb[:], _bcast_ap(zinv[:], 64))
                    osl = ohat[r0:r0 + 64, g0:g0 + 2, wcol:wcol + NTOK]
                    nc.vector.tensor_tensor(
                        out=osl,
                        in0=psO[0:64, :].rearrange("p (g t) -> p g t", g=2),
                        in1=zb[:].rearrange("p (g t) -> p g t", g=2),
                        op=OP.mult)
                    if vbc is not None:
                        for cg, hh in ((0, h0), (1, h1)):
                            nc.vector.tensor_scalar_add(
                                out=ohat[r0:r0 + 64, g0 + cg, wcol:wcol + NTOK],
                                in0=ohat[r0:r0 + 64, g0 + cg, wcol:wcol + NTOK],
                                scalar1=vbc[r0:r0 + 64, (hh // 2):(hh // 2) + 1])

        # ---------- proj + residual ----------
        x1 = x1pool.tile([P, KT, PC2], F32, tag="x1")
        for m in range(KT):
            pps = ps_mm.tile([P, PC2], F32, tag="mm")
            for k in range(KT):
                nc.tensor.matmul(pps[:, :pc], lhsT=wp[:, k, m * P:(m + 1) * P],
                                 rhs=ohat[:, k, :pc], start=(k == 0), stop=(k == KT - 1))
            nc.vector.scalar_tensor_tensor(out=x1[:, m, :pc], in0=pps[:, :pc],
                                           scalar=sc(pb, m), in1=xb[:, m, :pc],
                                           op0=OP.add, op1=OP.add)

        # ---------- LN2 + SwiGLU MLP ----------
        sxp2, sqp2 = ln_stats(x1, pc)
        A2, CA2, Ab2 = ln_tail(sxp2, sqp2, DIM, pc, "ab2")
        x1A = bfpool.tile([P, KT, PC2], BF16, tag="x1A")
        for k in range(KT):
            nc.vector.tensor_tensor(out=x1A[:, k, :pc], in0=x1[:, k, :pc],
                                    in1=Ab2[:, :pc], op=OP.mult)

        g = gpool.tile([P, MT, PC2], BF16, tag="g")
        sgp = ps_stat.tile([1, PC2], F32, tag="srow")
        ssgp = ps_stat.tile([1, PC2], F32, tag="srow")
        for m in range(MT):
            p1 = ps_mm.tile([P, PC2], F32, tag="mm")
            for k in range(KT):
                nc.tensor.matmul(p1[:, :pc], lhsT=w1[:, k, m * P:(m + 1) * P],
                                 rhs=x1A[:, k, :pc], start=(k == 0), stop=False)
            nc.tensor.matmul(p1[:, :pc], lhsT=nw1s[0:1, m * P:(m + 1) * P],
                             rhs=CA2[:, :pc], start=False, stop=True)
            s1 = mlppool.tile([P, PC2], BF16, tag="s1")
            nc.scalar.activation(out=s1[:, :pc], in_=p1[:, :pc], func=AF.Silu,
                                 bias=w1b[:, m:m + 1] if w1b is not None else zcol[:],
                                 scale=1.0)
            p2 = ps_mm.tile([P, PC2], F32, tag="mm")
            for k in range(KT):
                nc.tensor.matmul(p2[:, :pc], lhsT=w2[:, k, m * P:(m + 1) * P],
                                 rhs=x1A[:, k, :pc], start=(k == 0), stop=False)
            nc.tensor.matmul(p2[:, :pc], lhsT=nw2s[0:1, m * P:(m + 1) * P],
                             rhs=CA2[:, :pc], start=False, stop=True)
            nc.vector.scalar_tensor_tensor(out=g[:, m, :pc], in0=p2[:, :pc],
                                           scalar=sc(w2b, m), in1=s1[:, :pc],
                                           op0=OP.add, op1=OP.mult)
            sqt = mlppool.tile([P, PC2], BF16, tag="sqt")
            nc.scalar.activation(out=sqt[:, :pc], in_=g[:, m, :pc],
                                 func=AF.Square, bias=zcol[:], scale=1.0)
            nc.tensor.matmul(sgp[:, :pc], lhsT=ones[:, 0:1], rhs=g[:, m, :pc],
                             start=(m == 0), stop=(m == MT - 1), skip_group_check=True)
            nc.tensor.matmul(ssgp[:, :pc], lhsT=ones[:, 0:1], rhs=sqt[:, :pc],
                             start=(m == 0), stop=(m == MT - 1), skip_group_check=True)

        # ---------- hid-LN tail + w3 + residual -> output ----------
        A3, _, Ab3 = ln_tail(sgp, ssgp, HID, pc, "ab3", want_ca=False)
        c3raw = rowpool.tile([1, PC2], BF16, tag="c3raw")
        nc.scalar.activation(out=c3raw[:, :pc], in_=sgp[:, :pc],
                             func=AF.Copy, bias=0.0, scale=1.0)
        for m in range(KT):
            w3t = w3pool.tile([P, MT, P], BF16, tag="w3t")
            nc.sync.dma_start(w3t[:], w3d[:, :, m * P:(m + 1) * P])
            psW = ps_mm.tile([P, PC2], F32, tag="mm")
            for k in range(MT):
                nc.tensor.matmul(psW[:, :pc], lhsT=w3t[:, k, :],
                                 rhs=g[:, k, :pc], start=(k == 0), stop=False)
            nc.tensor.matmul(psW[:, :pc], lhsT=nw3s[0:1, m * P:(m + 1) * P],
                             rhs=c3raw[:, :pc], start=False, stop=True)
            yt = ypool.tile([P, PC2], F32, tag="yt")
            nc.vector.tensor_tensor(out=yt[:, :pc], in0=psW[:, :pc],
                                    in1=Ab3[:, :pc], op=OP.mult)
            nc.vector.scalar_tensor_tensor(out=yt[:, :pc], in0=yt[:, :pc],
                                           scalar=sc(w3b, m), in1=x1[:, m, :pc],
                                           op0=OP.add, op1=OP.add)
            nc.sync.dma_start(yT[:, m, c0:c0 + pc], yt[:, :pc])

    def emit_all_pairs():
        for wins in pairs:
            emit_pair(wins)

    if loop_n > 1:
        with tc.For_i(0, loop_n, 1):
            emit_all_pairs()
    else:
        emit_all_pairs()


def _build(has_biases, nwin_total=NWIN, ncores=N_CORES, loop_n=1):
    key = ("prog", tuple(sorted(has_biases.items())), nwin_total, ncores, loop_n)
    if key in _cache:
        return _cache[key]
    nc = bacc.Bacc("TRN2", target_bir_lowering=False, debug=False,
                   enable_asserts=False, num_devices=ncores)
    toks = nwin_total * NTOK
    aps = {}
    aps["xTb"] = nc.dram_tensor("xTb", [DIM, toks], BF16, kind="ExternalInput").ap()
    aps["yT"] = nc.dram_tensor("yT", [DIM, toks], F32, kind="ExternalOutput").ap()
    for nm, shp in [("wq", [DIM, DIM]), ("wk", [DIM, DIM]), ("wv", [DIM, DIM]),
                    ("wp", [DIM, DIM]), ("w1", [DIM, HID]), ("w2", [DIM, HID]),
                    ("w3", [HID, DIM])]:
        aps[nm] = nc.dram_tensor(nm, shp, BF16, kind="ExternalInput").ap()
    for nm, n in [("nqs", DIM), ("nks", DIM), ("nvs", DIM),
                  ("nw1s", HID), ("nw2s", HID), ("nw3s", DIM)]:
        aps[nm] = nc.dram_tensor(nm, [1, n], BF16, kind="ExternalInput").ap()
    aps["cos2"] = nc.dram_tensor("cos2", [P, PC2], BF16, kind="ExternalInput").ap()
    aps["sin2"] = nc.dram_tensor("sin2", [P, PC2], BF16, kind="ExternalInput").ap()
    aps["r2t"] = nc.dram_tensor("r2t", [P, P], BF16, kind="ExternalInput").ap()
    aps["ident"] = nc.dram_tensor("ident", [P, P], BF16, kind="ExternalInput").ap()
    # bias rows (bf16, applied via PE rank-1 matmuls) and bias cols (fp32)
    for nm, n in [("qbr", DIM), ("kbr", DIM), ("vbr", DIM)]:
        if has_biases.get(nm):
            aps[nm] = nc.dram_tensor(nm, [1, n], BF16, kind="ExternalInput").ap()
        else:
            aps[nm] = None
    for nm, n in [("pb", DIM), ("w1b", HID), ("w2b", HID), ("w3b", DIM),
                  ("vbc", DIM)]:
        if has_biases.get(nm):
            aps[nm] = nc.dram_tensor(nm, [n], F32, kind="ExternalInput").ap()
        else:
            aps[nm] = None
    with tile.TileContext(nc) as tc:
        with ExitStack() as ctx:
            _emit(nc, tc, ctx, aps, has_biases, nwin_total, loop_n)
    nc.compile()
    _cache[key] = nc
    return nc


def _host_prep(inputs):
    f = {k: np.asarray(v, np.float32) if hasattr(v, "shape") else v
         for k, v in inputs.items()}
    scale = HD ** -0.5
    wq = f["ln1_w"][:, None] * f["q_w"] * scale
    wk = f["ln1_w"][:, None] * f["k_w"]
    wv = f["ln1_w"][:, None] * f["v_w"]
    qb = (f["ln1_b"] @ f["q_w"] + f["q_b"]) * scale
    kb = f["ln1_b"] @ f["k_w"]
    vb = f["ln1_b"] @ f["v_w"] + f["v_b"]
    wp = f["proj_w"]
    pb = f["proj_b"]
    w1 = f["ln2_w"][:, None] * f["w1_w"]
    w2 = f["ln2_w"][:, None] * f["w2_w"]
    w1b = f["ln2_b"] @ f["w1_w"] + f["w1_b"]
    w2b = f["ln2_b"] @ f["w2_w"] + f["w2_b"]
    w3 = f["ffn_w"][:, None] * f["w3_w"]
    w3b = f["ffn_b"] @ f["w3_w"] + f["w3_b"]

    cos, sin = _rope_tables()
    cosT = np.ascontiguousarray(cos.T)
    sinT = np.ascontiguousarray(sin.T)
    cos2 = np.tile(np.concatenate([cosT, cosT], 0), (1, 2))   # [128, 392]
    sin2 = np.tile(np.concatenate([sinT, sinT], 0), (1, 2))

    r = np.zeros((64, 64), np.float32)
    for i in range(32):
        r[2 * i, 2 * i + 1] = -1.0
        r[2 * i + 1, 2 * i] = 1.0
    r2 = np.zeros((128, 128), np.float32)
    r2[:64, :64] = r
    r2[64:, 64:] = r
    r2t = np.ascontiguousarray(r2.T)
    ident = np.eye(P, dtype=np.float32)

    x = f["x"]
    pad = (-H) % WS
    nw = (H + pad) // WS
    xp = np.pad(x, ((0, 0), (0, pad), (0, pad), (0, 0)))
    t = xp.reshape(B, nw, WS, nw, WS, DIM).transpose(0, 1, 3, 2, 4, 5).reshape(B, NWIN * NTOK, DIM)

    shared = {
        "wq": wq.astype(BF16NP), "wk": wk.astype(BF16NP), "wv": wv.astype(BF16NP),
        "wp": wp.astype(BF16NP), "w1": w1.astype(BF16NP), "w2": w2.astype(BF16NP),
        "w3": w3.astype(BF16NP),
        "nqs": (-wq.sum(0) / DIM)[None, :].astype(BF16NP),
        "nks": (-wk.sum(0) / DIM)[None, :].astype(BF16NP),
        "nvs": (-wv.sum(0) / DIM)[None, :].astype(BF16NP),
        "nw1s": (-w1.sum(0) / DIM)[None, :].astype(BF16NP),
        "nw2s": (-w2.sum(0) / DIM)[None, :].astype(BF16NP),
        "nw3s": (-w3.sum(0) / HID)[None, :].astype(BF16NP),
        "cos2": cos2.astype(BF16NP), "sin2": sin2.astype(BF16NP),
        "r2t": r2t.astype(BF16NP), "ident": ident.astype(BF16NP),
    }
    biases = {"qbr": qb, "kbr": kb, "vbr": vb, "pb": pb,
              "w1b": w1b, "w2b": w2b, "w3b": w3b, "vbc": vb}
    has_biases = {k: bool(np.any(v != 0.0)) for k, v in biases.items()}
    # vbr (PE rank-1 path) is unused; vbc (per-head scalar add) is used.
    has_biases["vbr"] = False
    for k, v in biases.items():
        if has_biases[k]:
            if k in ("qbr", "kbr", "vbr"):
                shared[k] = np.ascontiguousarray(v, np.float32)[None, :].astype(BF16NP)
            else:
                shared[k] = np.ascontiguousarray(v, np.float32)
    in_maps = []
    for b in range(B):
        m = dict(shared)
        m["xTb"] = np.ascontiguousarray(t[b].T).astype(BF16NP)   # [768, 4900] bf16
        in_maps.append(m)
    return in_maps, has_biases


def _host_post(results):
    pad = (-H) % WS
    nw = (H + pad) // WS
    Hp = H + pad
    y = np.empty((B, H, W, DIM), np.float32)
    for b in range(B):
        yb = np.asarray(results[b]["yT"])
        yw = yb.T.reshape(nw, nw, WS, WS, DIM).transpose(0, 2, 1, 3, 4).reshape(Hp, Hp, DIM)
        y[b] = yw[:H, :W, :]
    return y


def kernel(**inputs):
    in_maps, has_biases = _host_prep(inputs)
    nc = _build(has_biases)
    res = run_bass_kernel_spmd(nc, in_maps, core_ids=list(range(N_CORES)))
    return _host_post(res.results)
